# revision 8
# baseline (speedup 1.0000x reference)
"""Trainium2 Bass kernel for nn_EulerMisorientation3D.

reference math (per voxel, Bunge ZXZ Euler angles scaled by [2pi, pi, 2pi]):
    g    = euler_to_matrix(x * scale)       (3x3 rotation)
    g_h  = euler_to_matrix(x_hat * scale)
    tr   = sum_i g_h[i,i] * inv(g)[i,i]     (inv(g) == g^T for rotations,
                                             diag(g^T) == diag(g))
    out  = mean( arccos(0.5*(tr-1))^2 )

Per-voxel closed form used here (alpha=2pi*x0, beta=pi*x1, gamma=2pi*x2):
    u  = cos(alpha+gamma)  v = cos(alpha-gamma)
    a  = cos^2(beta/2)     b = sin^2(beta/2)
    diag(g) = (u*a + v*b,  u*a - v*b,  a - b)
    1 + z = A2*(1+U2) + B2*(1+V2)
        with U2 = u*u_h, V2 = v*v_h, A2 = a*a_h, B2 = b*b_h, z = 0.5*(tr-1)
    theta = arccos(z) = 2*atan( sqrt((1-z)/(1+z)) )
          = pi/2 + 2*atan( tanh( 0.25*( ln(1-z) - ln(1+z) ) ) )
(ln/tanh/atan route: ACT Rsqrt/Reciprocal tables are banned in bass, and the
ScalarE arctan spline only accepts args in [-pi/2, pi/2]; the Gudermannian
form keeps the atan argument in (-1, 1).)

Sharding: the flattened voxel axis (2097152 voxels) is split evenly over the
8 NeuronCores; each core reduces its 262144 voxels to per-partition partial
sums which the host sums (fp64) and divides by N.
"""

import math

import numpy as np

import concourse.bass as bass
import concourse.bacc as bacc
import concourse.tile as tile
from concourse import mybir
from concourse.bass_utils import run_bass_kernel_spmd

F32 = mybir.dt.float32
AF = mybir.ActivationFunctionType
OP = mybir.AluOpType

N_CORES = 8
NVOX = 128 * 128 * 128          # 2097152 voxels
PER = NVOX // N_CORES           # 262144 voxels per core
P = 128                         # SBUF partitions
COLS = PER // P                 # 2048 free-dim columns per core
T = 4                           # tiles
FD = COLS // T                  # 512 columns per tile

PI = math.pi
LN_EPS = 5e-5                   # keeps ln() off <=0 from fp32 rounding


def build_bass(per=PER, t_tiles=T, fd=FD):
    nc = bacc.Bacc("TRN2", target_bir_lowering=False, debug=False,
                   num_devices=N_CORES)
    xs = nc.declare_dram_parameter("xs", [3, per], F32, isOutput=False)
    xh = nc.declare_dram_parameter("xh", [3, per], F32, isOutput=False)
    out = nc.declare_dram_parameter("o", [P, t_tiles], F32, isOutput=True)

    cols = per // P
    assert cols == t_tiles * fd

    xs_v = xs[:].rearrange("c (p q) -> p c q", p=P)
    xh_v = xh[:].rearrange("c (p q) -> p c q", p=P)

    with tile.TileContext(nc) as tc:
        with (
            tc.tile_pool(name="io", bufs=3) as io,
            tc.tile_pool(name="wk", bufs=2) as wk,
            tc.tile_pool(name="pq", bufs=t_tiles) as pq,
            tc.tile_pool(name="accp", bufs=1) as accp,
        ):
            acc = accp.tile([P, t_tiles], F32)
            # per-partition bias constants for ACT (bias must be an AP)
            b_mpi2 = accp.tile([P, 1], F32)
            b_eps = accp.tile([P, 1], F32)
            nc.vector.memset(b_mpi2, -PI / 2)
            nc.vector.memset(b_eps, LN_EPS)
            b_ppi2 = accp.tile([P, 1], F32)
            nc.vector.memset(b_ppi2, PI / 2)
            p4s, q4s, rs = [], [], []

            # ---- phase 1: trig (sin table set) down to P4/Q4 per tile ----
            for j in range(t_tiles):
                sl = slice(j * fd, (j + 1) * fd)
                xt = io.tile([P, 3, fd], F32, tag="xt")
                ht = io.tile([P, 3, fd], F32, tag="ht")
                nc.sync.dma_start(out=xt[:], in_=xs_v[:, :, sl])
                nc.sync.dma_start(out=ht[:], in_=xh_v[:, :, sl])

                sus, svs, sbs = [], [], []
                for name, src in (("x", xt), ("h", ht)):
                    x0 = src[:, 0, :]
                    x1 = src[:, 1, :]
                    x2 = src[:, 2, :]
                    # s = x0+x2 in [0,2); t = x0-x2 in (-1,1)
                    # m_u === s+0.25 (mod 1), m_v === t+0.25 (mod 1), both
                    # wrapped into [-0.5, 0.5] (add_range_wrap; DVE has no
                    # float mod).  sin(2pi*m) = cos(2pi*s_or_t), arg within
                    # the ACT sin spline domain (-4, 4).
                    pre_u = wk.tile([P, fd], F32, tag=f"pre_u{name}")
                    pre_v = wk.tile([P, fd], F32, tag=f"pre_v{name}")
                    nc.vector.scalar_tensor_tensor(
                        pre_u, x0, 0.25, x2, OP.add, OP.add)
                    nc.vector.scalar_tensor_tensor(
                        pre_v, x0, 1.25, x2, OP.add, OP.subtract)
                    m_u = wk.tile([P, fd], F32, tag=f"m_u{name}")
                    m_v = wk.tile([P, fd], F32, tag=f"m_v{name}")
                    nc.vector.add_range_wrap(m_u, pre_u, -1.0, 0.5, 1.0)
                    nc.vector.add_range_wrap(m_v, pre_v, -1.0, 0.5, 1.0)
                    su = wk.tile([P, fd], F32, tag=f"su{name}")
                    sv = wk.tile([P, fd], F32, tag=f"sv{name}")
                    sb = wk.tile([P, fd], F32, tag=f"sb{name}")
                    nc.scalar.activation(su, m_u, AF.Sin, bias=0.0, scale=2 * PI)
                    nc.scalar.activation(sv, m_v, AF.Sin, bias=0.0, scale=2 * PI)
                    # sb = sin(pi*x1 - pi/2) = -cos(pi*x1); a=(1-sb)/2 b=(1+sb)/2
                    nc.scalar.activation(sb, x1, AF.Sin, bias=b_mpi2[:], scale=PI)
                    sus.append(su)
                    svs.append(sv)
                    sbs.append(sb)

                u2 = wk.tile([P, fd], F32, tag="u2")
                v2 = wk.tile([P, fd], F32, tag="v2")
                nc.vector.tensor_mul(u2, sus[0], sus[1])   # = u*u_h
                nc.vector.tensor_mul(v2, svs[0], svs[1])   # = v*v_h

                sbh_m1 = wk.tile([P, fd], F32, tag="sbh_m1")
                sbh_p1 = wk.tile([P, fd], F32, tag="sbh_p1")
                nc.vector.tensor_scalar(sbh_m1, sbs[1], 1.0, None, OP.subtract)
                nc.vector.tensor_scalar(sbh_p1, sbs[1], 1.0, None, OP.add)
                a4 = wk.tile([P, fd], F32, tag="a4")
                b4 = wk.tile([P, fd], F32, tag="b4")
                # 4*a*a_h = (1-sb_x)(1-sb_h) ; 4*b*b_h = (1+sb_x)(1+sb_h)
                nc.vector.scalar_tensor_tensor(
                    a4, sbs[0], 1.0, sbh_m1, OP.subtract, OP.mult)
                nc.vector.scalar_tensor_tensor(
                    b4, sbs[0], 1.0, sbh_p1, OP.add, OP.mult)

                t1 = wk.tile([P, fd], F32, tag="t1")
                t2 = wk.tile([P, fd], F32, tag="t2")
                nc.vector.scalar_tensor_tensor(t1, u2, 1.0, a4, OP.add, OP.mult)
                nc.vector.scalar_tensor_tensor(t2, v2, 1.0, b4, OP.add, OP.mult)

                p4 = pq.tile([P, fd], F32, tag="p4")     # = 4*(1+z)
                q4 = pq.tile([P, fd], F32, tag="q4")     # = 4*(1-z)
                nc.vector.tensor_add(p4, t1, t2)
                nc.vector.tensor_scalar(q4, p4, -1.0, 8.0, OP.mult, OP.add)
                p4s.append(p4)
                q4s.append(q4)

            # ---- phase 2: ln/exp table set ----
            for j in range(t_tiles):
                l1 = wk.tile([P, fd], F32, tag="l1")
                l2 = wk.tile([P, fd], F32, tag="l2")
                nc.scalar.activation(l1, p4s[j], AF.Ln, bias=b_eps[:], scale=1.0)
                nc.scalar.activation(l2, q4s[j], AF.Ln, bias=b_eps[:], scale=1.0)
                dd = wk.tile([P, fd], F32, tag="dd")
                nc.vector.tensor_sub(dd, l2, l1)
                r = pq.tile([P, fd], F32, tag="r")
                # tanh(0.25*dd); theta = pi/2 + 2*atan(tanh(0.25*dd))
                nc.scalar.activation(r, dd, AF.Tanh, bias=0.0, scale=0.25)
                rs.append(r)

            # ---- phase 3: back to trig set: atan, square, row-sum ----
            for j in range(t_tiles):
                at = wk.tile([P, fd], F32, tag="at")
                nc.scalar.activation(at, rs[j], AF.Arctan)
                sq = wk.tile([P, fd], F32, tag="sq")
                # theta = pi/2 + 2*at; theta^2 = Square(2*at + pi/2); accum_out
                nc.scalar.activation(
                    sq, at, AF.Square, bias=b_ppi2[:], scale=2.0,
                    accum_out=acc[:, j:j + 1])

            nc.sync.dma_start(out=out[:], in_=acc[:])

    nc.compile()
    return nc


_CACHE = {}


def _get_nc():
    if "nc" not in _CACHE:
        _CACHE["nc"] = build_bass()
    return _CACHE["nc"]


def _run(x, x_hat, **spmd_kwargs):
    x = np.ascontiguousarray(np.asarray(x, dtype=np.float32).reshape(3, NVOX))
    xh = np.ascontiguousarray(np.asarray(x_hat, dtype=np.float32).reshape(3, NVOX))

    in_maps = []
    for c in range(N_CORES):
        sl = slice(c * PER, (c + 1) * PER)
        in_maps.append({
            "xs": np.ascontiguousarray(x[:, sl]),
            "xh": np.ascontiguousarray(xh[:, sl]),
        })

    nc = _get_nc()
    res = run_bass_kernel_spmd(
        nc, in_maps, core_ids=list(range(N_CORES)), **spmd_kwargs)
    total = 0.0
    for r in res.results:
        total += r["o"].astype(np.float64).sum()
    return np.float32(total / NVOX), res


def kernel(x: np.ndarray, x_hat: np.ndarray) -> np.ndarray:
    val, _ = _run(x, x_hat)
    return val
